# revision 40
# baseline (speedup 1.0000x reference)
"""Multi-head attention (B=2, L=2048, D=1024, H=16) on 8 TRN2 NeuronCores.

Sharding: core c handles batch b = c//4 and head group g = c%4 (4 heads,
256 features). Per core:
  - project q, k (feature-major) and v (row-major) in bf16
  - per (qh, hp) group and query-half h (512 cols):
      scores^T tile per m: both heads in one [128,1024] PSUM tile
      (hi0 -> cols 0:512 via PE row-tile bp0, hi1 -> cols 512:1024 via bp64,
      concurrent in the array)
      exp: split between ScalarE (table exp, bf16 out) and VectorE
      (Schraudolph int16 bit-trick, bitcast to bf16) to double exp throughput
      ctx chains: col-tiled pairs, 2 heads share one [128,512] PSUM bank
      denominators: M=1 ones-matmul chains at col positions 0/32/64/96
      normalize: [33,512] contiguous copy + reciprocal + K=33 selector
      matmul broadcast + DVE multiply
  - output projection out^T = Wo_h ctx^T, interleaved into the next
    qh's m-loop to keep ScalarE busy
Host sums the 4 per-head-group partials per batch and adds bo.
"""

import math
import os
import sys

sys.path.insert(0, "/opt/trn_rl_repo")

import ml_dtypes
import numpy as np

import concourse.bass as bass
import concourse.mybir as mybir
import concourse.tile as tile
from concourse import bacc
from concourse.bass_utils import run_bass_kernel_spmd

B, L, D, H, DH = 2, 2048, 1024, 16, 64
NCORES = 8
HPC = 4                  # heads per core
FPC = HPC * DH           # 256 features per core
ND = D // 128            # 8 contraction tiles
NFT = FPC // 128         # 2 feature tiles for q/k/ctx
NM = L // 128            # 16 key tiles
SCALE = 1.0 / math.sqrt(DH)
CDT = mybir.dt.bfloat16
NP_CDT = ml_dtypes.bfloat16
F32 = mybir.dt.float32
I16 = mybir.dt.int16
F16 = mybir.dt.float16
EXP = mybir.ActivationFunctionType.Exp
MULT = mybir.AluOpType.mult
ADD = mybir.AluOpType.add
OUT_NAME = "outT"

# Schraudolph exp in bf16 bit-space: bf16(i16(x*A16 + B16)) ~ exp(x*SCALE)
LOG2E = 1.4426950408889634
A16 = float(np.float32(SCALE * 128.0 * LOG2E))
B16 = float(np.float32((127 << 7) - 5.5908))

# which (h, m) exp tiles go to the Vector engine (rest on Scalar)
DVE_FRAC_MOD = int(os.environ.get("DVE_MOD", "6"))  # m % MOD == 1 -> DVE; 0 = none
DEN_PAIR = os.environ.get("DEN_PAIR", "0") == "1"
POPS = os.environ.get("POPS", "1") == "1"
LAG = int(os.environ.get("LAG", "3"))

_CACHE = {}


def _dve_tile(m):
    return DVE_FRAC_MOD > 0 and (m % DVE_FRAC_MOD) == 1


def build_nc():
    nc = bacc.Bacc(
        "TRN2",
        target_bir_lowering=False,
        debug=False,
        enable_asserts=False,
        num_devices=NCORES,
    )
    xqT_d = nc.dram_tensor("xqT", [D, L], CDT, kind="ExternalInput")
    xkT_d = nc.dram_tensor("xkT", [D, L], CDT, kind="ExternalInput")
    xvT_d = nc.dram_tensor("xvT", [D, L], CDT, kind="ExternalInput")
    wq_d = nc.dram_tensor("wqT", [D, FPC], CDT, kind="ExternalInput")
    wk_d = nc.dram_tensor("wkT", [D, FPC], CDT, kind="ExternalInput")
    wv_d = nc.dram_tensor("wvT", [D, FPC], CDT, kind="ExternalInput")
    wo_d = nc.dram_tensor("woT", [FPC, D], CDT, kind="ExternalInput")
    bq_d = nc.dram_tensor("bq2", [128, NFT], F32, kind="ExternalInput")
    bk_d = nc.dram_tensor("bk2", [128, NFT], F32, kind="ExternalInput")
    bvb_d = nc.dram_tensor("bvb", [128, FPC], F32, kind="ExternalInput")
    out_d = nc.dram_tensor(OUT_NAME, [D, L], F16, kind="ExternalOutput")

    with tile.TileContext(nc) as tc:
        with tc.tile_pool(name="persist", bufs=1) as pp:
            qT = pp.tile([128, NFT, L], CDT)
            kT = pp.tile([128, NFT, L], CDT)
            vsb = pp.tile([128, NM, FPC], CDT)
            ctxT = pp.tile([128, NFT, L], CDT)
            wo_sb = pp.tile([128, NFT, D], CDT)
            bq_sb = pp.tile([128, NFT], F32)
            bk_sb = pp.tile([128, NFT], F32)
            bvb_sb = pp.tile([128, FPC], F32)
            ones_sb = pp.tile([128, 1], CDT)
            sel33 = pp.tile([33, 128], F32)
            d33p = pp.tile([33, 512], F32)

            nc.vector.memset(ones_sb[:], 1.0)
            nc.vector.memset(d33p[:], 1.0)
            nc.vector.memset(sel33[:], 0.0)
            nc.vector.memset(sel33[0:1, 0:64], 1.0)
            nc.vector.memset(sel33[32:33, 64:128], 1.0)
            # weights/biases not needed until later go via the GpSimd DGE
            nc.gpsimd.dma_start(bq_sb[:], bq_d[:])
            nc.gpsimd.dma_start(bk_sb[:], bk_d[:])
            nc.gpsimd.dma_start(bvb_sb[:], bvb_d[:])
            nc.gpsimd.dma_start(
                wo_sb[:], wo_d.rearrange("(n p) f -> p n f", p=128)
            )

            # ---- Phase A: k/q projections (v is carried into phase B) ----
            with tc.tile_pool(name="phA", bufs=1) as pa:
                wq_sb = pa.tile([128, ND, FPC], CDT)
                wk_sb = pa.tile([128, ND, FPC], CDT)
                wv_sb = pa.tile([128, ND, FPC], CDT)
                xq_sb = pa.tile([128, ND, L], CDT)
                xk_sb = pa.tile([128, ND, L], CDT)
                xv_sb = pa.tile([128, ND, L], CDT)
                xq_r = xqT_d.rearrange("(n p) l -> p n l", p=128)
                xk_r = xkT_d.rearrange("(n p) l -> p n l", p=128)
                xv_r = xvT_d.rearrange("(n p) l -> p n l", p=128)
                wq_r = wq_d.rearrange("(n p) f -> p n f", p=128)
                wk_r = wk_d.rearrange("(n p) f -> p n f", p=128)
                wv_r = wv_d.rearrange("(n p) f -> p n f", p=128)
                for d in range(ND):
                    nc.sync.dma_start(wk_sb[:, d, :], wk_r[:, d, :])
                    nc.sync.dma_start(xk_sb[:, d, :], xk_r[:, d, :])
                for d in range(ND):
                    nc.sync.dma_start(wq_sb[:, d, :], wq_r[:, d, :])
                    nc.sync.dma_start(xq_sb[:, d, :], xq_r[:, d, :])
                for d in range(ND):
                    nc.sync.dma_start(wv_sb[:, d, :], wv_r[:, d, :])
                    nc.sync.dma_start(xv_sb[:, d, :], xv_r[:, d, :])

                with tc.tile_pool(name="psA", bufs=4, space="PSUM") as psA:

                    def proj_qk(x_sb, w_sb, b_sb, dstT, ti):
                        for ft in range(NFT):
                            pss = [
                                psA.tile(
                                    [128, 512], F32, tag="projqk",
                                    name=f"pjk_{ti}_{ft}_{ch}",
                                )
                                for ch in range(4)
                            ]
                            for d in range(ND):
                                for ch in range(4):
                                    nc.tensor.matmul(
                                        pss[ch][:],
                                        w_sb[:, d, ft * 128 : (ft + 1) * 128],
                                        x_sb[:, d, ch * 512 : (ch + 1) * 512],
                                        start=(d == 0),
                                        stop=(d == ND - 1),
                                    )
                            for ch in range(4):
                                nc.vector.tensor_scalar_add(
                                    dstT[:, ft, ch * 512 : (ch + 1) * 512],
                                    pss[ch][:],
                                    b_sb[:, ft : ft + 1],
                                )

                    proj_qk(xk_sb, wk_sb, bk_sb, kT, 1)
                    proj_qk(xq_sb, wq_sb, bq_sb, qT, 0)

                def make_vproj(pool):
                    # v projection kt-chains: row-major [keys, 256 feats]
                    def vproj(kt):
                        ops = pool.tile(
                            [128, 512], F32, tag="acc512", name=f"vp_{kt}"
                        )
                        for d in range(ND):
                            nc.tensor.matmul(
                                ops[:, 0:FPC],
                                xv_sb[:, d, kt * 128 : (kt + 1) * 128],
                                wv_sb[:, d, :],
                                start=(d == 0),
                                stop=(d == ND - 1),
                            )
                        nc.vector.tensor_add(
                            vsb[:, kt, :], ops[:, 0:FPC], bvb_sb[:]
                        )
                    return vproj

                kphase = os.environ.get("KPHASE", "full")
                if kphase == "A":
                    with (
                        tc.tile_pool(name="dbg", bufs=2) as dbg,
                        tc.tile_pool(name="psdbg", bufs=2, space="PSUM") as psd,
                    ):
                        vp = make_vproj(psd)
                        for kt in range(NM):
                            vp(kt)
                        for ft in range(NFT):
                            st = dbg.tile([128, L], F16, tag="st")
                            nc.vector.tensor_copy(st[:], qT[:, ft, :])
                            nc.sync.dma_start(
                                out_d[ft * 128 : (ft + 1) * 128, :], st[:]
                            )
                            st2 = dbg.tile([128, L], F16, tag="st")
                            nc.vector.tensor_copy(st2[:], kT[:, ft, :])
                            nc.sync.dma_start(
                                out_d[(2 + ft) * 128 : (3 + ft) * 128, :], st2[:]
                            )
                            st3 = dbg.tile([128, L], F16, tag="st")
                            nc.vector.tensor_copy(
                                st3[:], vsb[:, ft * 8 : ft * 8 + 8, :]
                            )
                            nc.sync.dma_start(
                                out_d[(4 + ft) * 128 : (5 + ft) * 128, :], st3[:]
                            )
                if kphase in ("AB", "full"):
                    phase_b(
                        nc, tc, qT, kT, vsb, ctxT, wo_sb, ones_sb, sel33,
                        d33p, out_d, kphase, make_vproj,
                    )
    nc.compile()
    return nc


def phase_b(
    nc, tc, qT, kT, vsb, ctxT, wo_sb, ones_sb, sel33, d33p, out_d, kphase,
    make_vproj,
):
    with (
        tc.tile_pool(name="probs", bufs=8) as pb,
        tc.tile_pool(name="prsum", bufs=4) as pbs,
        tc.tile_pool(name="smalls", bufs=3) as sm,
        tc.tile_pool(name="psS", bufs=2, space="PSUM") as psS,
        tc.tile_pool(name="psC", bufs=2, space="PSUM") as psC,
        tc.tile_pool(name="psD", bufs=1, space="PSUM") as psD,
        tc.tile_pool(name="psX", bufs=1, space="PSUM") as psX,
    ):
        carried = []  # deferred output-projection work items (closures)
        vproj = make_vproj(psX)

        def emit_outproj_lc(lc):
            # out^T chunk [128, 512] per ft8; contract ctxT over NFT
            for ft8 in range(D // 128):
                def work(lc=lc, ft8=ft8, pool=psX):
                    tag = "acc512" if pool is psX else (
                        "den" if pool is psD else "ctx"
                    )
                    ops = pool.tile(
                        [128, 512], F32, tag=tag, name=f"op_{lc}_{ft8}",
                    )
                    for d2 in range(NFT):
                        nc.tensor.matmul(
                            ops[:],
                            wo_sb[:, d2, ft8 * 128 : (ft8 + 1) * 128],
                            ctxT[:, d2, lc * 512 : (lc + 1) * 512],
                            start=(d2 == 0),
                            stop=(d2 == NFT - 1),
                        )
                    st = sm.tile(
                        [128, 512], F16, tag="ost", bufs=4,
                        name=f"st_{lc}_{ft8}",
                    )
                    nc.vector.tensor_copy(st[:], ops[:])
                    eng = nc.gpsimd if ft8 % 2 == 0 else nc.sync
                    eng.dma_start(
                        out_d[
                            ft8 * 128 : (ft8 + 1) * 128,
                            lc * 512 : (lc + 1) * 512,
                        ],
                        st[:],
                    )
                carried.append(work)

        for qh in range(2):
            for hp in range(2):
                den = psD.tile([128, 512], F32, tag="den", name=f"den_{qh}_{hp}")
                g0 = hp * 2  # head index base within this core's 4 heads
                for h in range(2):
                    ctx = psC.tile(
                        [128, 512], F32, tag="ctx", name=f"ctx_{qh}_{hp}_{h}"
                    )
                    q0 = qh * 1024 + h * 512
                    dp = 64 * h  # den partition base for this half
                    probs = {}
                    prsums = {}

                    def attn_consume(m, ctx=ctx, dp=dp, g0=g0, probs=probs):
                        pr = probs[m]
                        if not DEN_PAIR:
                            # den chains first: their 1-col weight load is
                            # free, letting the ctx chains' loads overlap them
                            for hi in range(2):
                                dr = dp + hi * 32
                                nc.tensor.matmul(
                                    den[dr : dr + 1, :],
                                    ones_sb[:],
                                    pr[:, hi * 512 : (hi + 1) * 512],
                                    start=(m == 0),
                                    stop=(m == NM - 1),
                                    tile_position=(0, dr),
                                )
                        # ctx chains: col-tiled pair, 2 heads in one bank
                        for hi in range(2):
                            nc.tensor.matmul(
                                ctx[hi * 64 : (hi + 1) * 64, :],
                                vsb[:, m, (g0 + hi) * 64 : (g0 + hi + 1) * 64],
                                pr[:, hi * 512 : (hi + 1) * 512],
                                start=(m == 0),
                                stop=(m == NM - 1),
                            )

                    def den_pair(p, dp=dp, prsums=prsums):
                        # denominator chains over pair-sums: M=1 cols dp, dp+32
                        ps = prsums.pop(p)
                        for hi in range(2):
                            dr = dp + hi * 32
                            nc.tensor.matmul(
                                den[dr : dr + 1, :],
                                ones_sb[:],
                                ps[:, hi * 512 : (hi + 1) * 512],
                                start=(p == 0),
                                stop=(p == NM // 2 - 1),
                                tile_position=(0, dr),
                            )

                    for m in range(NM):
                        sc = psS.tile(
                            [128, 1024], F32, tag="sc",
                            name=f"sc_{qh}_{hp}_{h}_{m}",
                        )
                        # scores: both heads, row-tiled pair (bp0 / bp64)
                        for hi in range(2):
                            po = hi * 64
                            nc.tensor.matmul(
                                sc[:, hi * 512 : (hi + 1) * 512],
                                kT[po : po + 64, hp, m * 128 : (m + 1) * 128],
                                qT[po : po + 64, hp, q0 : q0 + 512],
                                start=True,
                                stop=True,
                            )
                        # exp -> probs bf16 (both heads in one tile)
                        pr = pb.tile(
                            [128, 1024], CDT, tag="pr",
                            name=f"pr_{qh}_{hp}_{h}_{m}",
                        )
                        if _dve_tile(m):
                            nc.vector.tensor_scalar(
                                pr[:].bitcast(I16), sc[:], A16, B16, MULT, ADD
                            )
                        else:
                            nc.scalar.activation(pr[:], sc[:], EXP, scale=SCALE)
                        probs[m] = pr
                        # pair-sum probs on the idle GpSimd engine so the den
                        # matmul stream halves (den rides the pair sums)
                        if DEN_PAIR and m % 2 == 1:
                            pst = pbs.tile(
                                [128, 1024], CDT, tag="prs",
                                name=f"prs_{qh}_{hp}_{h}_{m}",
                            )
                            nc.gpsimd.tensor_add(
                                pst[:], probs[m - 1][:], probs[m][:]
                            )
                            prsums[m // 2] = pst
                        # software pipeline: consume probs LAG slots back so
                        # the PE never waits on exp (sc tiles are freed by exp,
                        # not by the chains, so this costs no PSUM)
                        if m >= LAG:
                            attn_consume(m - LAG)
                            probs.pop(m - LAG)
                        if DEN_PAIR and m >= 4 and m % 2 == 0:
                            den_pair(m // 2 - 2)
                        # carried work: v-projection (first group) feeds
                        # chains just in time (chains(m) run at slot m+LAG);
                        # outproj drains at odd m
                        if qh == 0 and hp == 0 and h == 0 and m >= 2:
                            vproj(m - 2)
                        elif POPS and carried and (m % 2) == 1:
                            carried.pop(0)()
                    if qh == 0 and hp == 0 and h == 0:
                        vproj(NM - 2)
                        vproj(NM - 1)
                    for j in range(NM - LAG, NM):
                        attn_consume(j)
                        probs.pop(j)
                    if DEN_PAIR:
                        den_pair(NM // 2 - 2)
                        den_pair(NM // 2 - 1)
                    # ---- normalize this half ----
                    # d33p rows 1-31 are memset to 1.0; only rows 0/32 carry
                    # dens, so recip never makes Inf/NaN that the selector
                    # matmul would 0*Inf-poison
                    nc.vector.tensor_copy(d33p[0:1, :], den[dp : dp + 1, :])
                    nc.vector.tensor_copy(
                        d33p[32:33, :], den[dp + 32 : dp + 33, :]
                    )
                    r33 = sm.tile(
                        [33, 512], F32, tag="r33", name=f"r33_{qh}_{hp}_{h}"
                    )
                    nc.vector.reciprocal_approx_fast(r33[:], d33p[:])
                    rb = psX.tile(
                        [128, 512], F32, tag="acc512", name=f"rb_{qh}_{hp}_{h}"
                    )
                    nc.tensor.matmul(rb[:], sel33[:], r33[:], start=True, stop=True)
                    rb_sb = sm.tile(
                        [128, 512], F32, tag="rbsb", name=f"rbsb_{qh}_{hp}_{h}"
                    )
                    nc.scalar.copy(rb_sb[:], rb[:])
                    nc.vector.tensor_mul(
                        ctxT[:, hp, q0 : q0 + 512], ctx[:], rb_sb[:]
                    )
                    # outproj chunk lc=2qh+h is complete once both hp groups
                    # normalized this half's columns
                    if hp == 1 and kphase != "AB":
                        emit_outproj_lc(qh * 2 + h)
        if kphase == "AB":
            for ft in range(NFT):
                st = sm.tile([128, L], F16, tag="ostage", bufs=1, name=f"dbg_{ft}")
                nc.vector.tensor_copy(st[:], ctxT[:, ft, :])
                nc.sync.dma_start(out_d[ft * 128 : (ft + 1) * 128, :], st[:])
            return
        # tail: remaining deferred outproj work, rotate over 4 free banks
        pools = [psX, psD, psC, psC]
        for i, work in enumerate(carried):
            work.__defaults__ = (
                work.__defaults__[0],
                work.__defaults__[1],
                pools[i % 4],
            )
            work()
        carried.clear()


def make_in_maps(Q, K, V, Wq, bq, Wk, bk, Wv, bv, Wo, bo):
    Q = np.asarray(Q, np.float32)
    K = np.asarray(K, np.float32)
    V = np.asarray(V, np.float32)
    xqT = [np.ascontiguousarray(Q[b].T).astype(NP_CDT) for b in range(B)]
    xkT = [np.ascontiguousarray(K[b].T).astype(NP_CDT) for b in range(B)]
    xvT = [np.ascontiguousarray(V[b].T).astype(NP_CDT) for b in range(B)]
    in_maps = []
    for c in range(NCORES):
        b, g = divmod(c, HPC)
        fs = slice(g * FPC, (g + 1) * FPC)
        wqT = np.ascontiguousarray(np.asarray(Wq, np.float32)[fs, :].T).astype(NP_CDT)
        wkT = np.ascontiguousarray(np.asarray(Wk, np.float32)[fs, :].T).astype(NP_CDT)
        wvT = np.ascontiguousarray(np.asarray(Wv, np.float32)[fs, :].T).astype(NP_CDT)
        woT = np.ascontiguousarray(np.asarray(Wo, np.float32)[:, fs].T).astype(NP_CDT)
        bq2 = np.ascontiguousarray(
            np.asarray(bq, np.float32)[fs].reshape(NFT, 128).T
        )
        bk2 = np.ascontiguousarray(
            np.asarray(bk, np.float32)[fs].reshape(NFT, 128).T
        )
        bv_blk = np.asarray(bv, np.float32)[fs]
        in_maps.append(
            {
                "xqT": xqT[b],
                "xkT": xkT[b],
                "xvT": xvT[b],
                "wqT": wqT,
                "wkT": wkT,
                "wvT": wvT,
                "woT": woT,
                "bq2": bq2,
                "bk2": bk2,
                "bvb": np.broadcast_to(bv_blk, (128, FPC)).copy(),
            }
        )
    return in_maps


def assemble(results, bo):
    out = np.zeros((B, L, D), np.float32)
    for c in range(NCORES):
        b = c // HPC
        out[b] += results[c][OUT_NAME].T.astype(np.float32)
    out += np.asarray(bo, np.float32)[None, None, :]
    return out


def kernel(Q, K, V, Wq, bq, Wk, bk, Wv, bv, Wo, bo):
    if "nc" not in _CACHE:
        _CACHE["nc"] = build_nc()
    nc = _CACHE["nc"]
    in_maps = make_in_maps(Q, K, V, Wq, bq, Wk, bk, Wv, bv, Wo, bo)
    res = run_bass_kernel_spmd(nc, in_maps, core_ids=list(range(NCORES)))
    return assemble(res.results, bo)


# revision 41
# speedup vs baseline: 1.0062x; 1.0062x over previous
"""Multi-head attention (B=2, L=2048, D=1024, H=16) on 8 TRN2 NeuronCores.

Sharding: core c handles batch b = c//4 and head group g = c%4 (4 heads,
256 features). Per core:
  - project q, k (feature-major) and v (row-major) in bf16
  - per (qh, hp) group and query-half h (512 cols):
      scores^T tile per m: both heads in one [128,1024] PSUM tile
      (hi0 -> cols 0:512 via PE row-tile bp0, hi1 -> cols 512:1024 via bp64,
      concurrent in the array)
      exp: split between ScalarE (table exp, bf16 out) and VectorE
      (Schraudolph int16 bit-trick, bitcast to bf16) to double exp throughput
      ctx chains: col-tiled pairs, 2 heads share one [128,512] PSUM bank
      denominators: M=1 ones-matmul chains at col positions 0/32/64/96
      normalize: [33,512] contiguous copy + reciprocal + K=33 selector
      matmul broadcast + DVE multiply
  - output projection out^T = Wo_h ctx^T, interleaved into the next
    qh's m-loop to keep ScalarE busy
Host sums the 4 per-head-group partials per batch and adds bo.
"""

import math
import os
import sys

sys.path.insert(0, "/opt/trn_rl_repo")

import ml_dtypes
import numpy as np

import concourse.bass as bass
import concourse.mybir as mybir
import concourse.tile as tile
from concourse import bacc
from concourse.bass_utils import run_bass_kernel_spmd

B, L, D, H, DH = 2, 2048, 1024, 16, 64
NCORES = 8
HPC = 4                  # heads per core
FPC = HPC * DH           # 256 features per core
ND = D // 128            # 8 contraction tiles
NFT = FPC // 128         # 2 feature tiles for q/k/ctx
NM = L // 128            # 16 key tiles
SCALE = 1.0 / math.sqrt(DH)
CDT = mybir.dt.bfloat16
NP_CDT = ml_dtypes.bfloat16
F32 = mybir.dt.float32
I16 = mybir.dt.int16
F16 = mybir.dt.float16
EXP = mybir.ActivationFunctionType.Exp
MULT = mybir.AluOpType.mult
ADD = mybir.AluOpType.add
OUT_NAME = "outT"

# Schraudolph exp in bf16 bit-space: bf16(i16(x*A16 + B16)) ~ exp(x*SCALE)
LOG2E = 1.4426950408889634
A16 = float(np.float32(SCALE * 128.0 * LOG2E))
B16 = float(np.float32((127 << 7) - 5.5908))

# which (h, m) exp tiles go to the Vector engine (rest on Scalar)
DVE_FRAC_MOD = int(os.environ.get("DVE_MOD", "6"))  # m % MOD == 1 -> DVE; 0 = none
DEN_PAIR = os.environ.get("DEN_PAIR", "0") == "1"
POPS = os.environ.get("POPS", "1") == "1"
LAG = int(os.environ.get("LAG", "3"))

_CACHE = {}


def _dve_tile(m):
    return DVE_FRAC_MOD > 0 and (m % DVE_FRAC_MOD) == 1


def build_nc():
    nc = bacc.Bacc(
        "TRN2",
        target_bir_lowering=False,
        debug=False,
        enable_asserts=False,
        num_devices=NCORES,
    )
    xqT_d = nc.dram_tensor("xqT", [D, L], CDT, kind="ExternalInput")
    xkT_d = nc.dram_tensor("xkT", [D, L], CDT, kind="ExternalInput")
    xvT_d = nc.dram_tensor("xvT", [D, L], CDT, kind="ExternalInput")
    wq_d = nc.dram_tensor("wqT", [D, FPC], CDT, kind="ExternalInput")
    wk_d = nc.dram_tensor("wkT", [D, FPC], CDT, kind="ExternalInput")
    wv_d = nc.dram_tensor("wvT", [D, FPC], CDT, kind="ExternalInput")
    wo_d = nc.dram_tensor("woT", [FPC, D], CDT, kind="ExternalInput")
    bq_d = nc.dram_tensor("bq2", [128, NFT], F32, kind="ExternalInput")
    bk_d = nc.dram_tensor("bk2", [128, NFT], F32, kind="ExternalInput")
    bvb_d = nc.dram_tensor("bvb", [128, FPC], F32, kind="ExternalInput")
    out_d = nc.dram_tensor(OUT_NAME, [D, L], F16, kind="ExternalOutput")

    with tile.TileContext(nc) as tc:
        with tc.tile_pool(name="persist", bufs=1) as pp:
            qT = pp.tile([128, NFT, L], CDT)
            kT = pp.tile([128, NFT, L], CDT)
            vsb = pp.tile([128, NM, FPC], CDT)
            ctxT = pp.tile([128, NFT, L], CDT)
            wo_sb = pp.tile([128, NFT, D], CDT)
            bq_sb = pp.tile([128, NFT], F32)
            bk_sb = pp.tile([128, NFT], F32)
            bvb_sb = pp.tile([128, FPC], F32)
            ones_sb = pp.tile([128, 1], CDT)
            sel33 = pp.tile([33, 128], F32)
            d33p = pp.tile([33, 512], F32)

            nc.vector.memset(ones_sb[:], 1.0)
            nc.vector.memset(d33p[:], 1.0)
            nc.vector.memset(sel33[:], 0.0)
            nc.vector.memset(sel33[0:1, 0:64], 1.0)
            nc.vector.memset(sel33[32:33, 64:128], 1.0)
            # weights/biases not needed until later go via the GpSimd DGE
            nc.gpsimd.dma_start(bq_sb[:], bq_d[:])
            nc.gpsimd.dma_start(bk_sb[:], bk_d[:])
            nc.gpsimd.dma_start(bvb_sb[:], bvb_d[:])
            nc.gpsimd.dma_start(
                wo_sb[:], wo_d.rearrange("(n p) f -> p n f", p=128)
            )

            # ---- Phase A: k/q projections (v is carried into phase B) ----
            with tc.tile_pool(name="phA", bufs=1) as pa:
                wq_sb = pa.tile([128, ND, FPC], CDT)
                wk_sb = pa.tile([128, ND, FPC], CDT)
                wv_sb = pa.tile([128, ND, FPC], CDT)
                xq_sb = pa.tile([128, ND, L], CDT)
                xk_sb = pa.tile([128, ND, L], CDT)
                xv_sb = pa.tile([128, ND, L], CDT)
                xq_r = xqT_d.rearrange("(n p) l -> p n l", p=128)
                xk_r = xkT_d.rearrange("(n p) l -> p n l", p=128)
                xv_r = xvT_d.rearrange("(n p) l -> p n l", p=128)
                wq_r = wq_d.rearrange("(n p) f -> p n f", p=128)
                wk_r = wk_d.rearrange("(n p) f -> p n f", p=128)
                wv_r = wv_d.rearrange("(n p) f -> p n f", p=128)
                for d in range(ND):
                    nc.sync.dma_start(wk_sb[:, d, :], wk_r[:, d, :])
                    nc.sync.dma_start(xk_sb[:, d, :], xk_r[:, d, :])
                for d in range(ND):
                    nc.sync.dma_start(wq_sb[:, d, :], wq_r[:, d, :])
                    nc.sync.dma_start(xq_sb[:, d, :], xq_r[:, d, :])
                for d in range(ND):
                    nc.sync.dma_start(wv_sb[:, d, :], wv_r[:, d, :])
                    nc.sync.dma_start(xv_sb[:, d, :], xv_r[:, d, :])

                with tc.tile_pool(name="psA", bufs=4, space="PSUM") as psA:

                    def proj_qk(x_sb, w_sb, b_sb, dstT, ti):
                        for ft in range(NFT):
                            pss = [
                                psA.tile(
                                    [128, 512], F32, tag="projqk",
                                    name=f"pjk_{ti}_{ft}_{ch}",
                                )
                                for ch in range(4)
                            ]
                            for d in range(ND):
                                for ch in range(4):
                                    nc.tensor.matmul(
                                        pss[ch][:],
                                        w_sb[:, d, ft * 128 : (ft + 1) * 128],
                                        x_sb[:, d, ch * 512 : (ch + 1) * 512],
                                        start=(d == 0),
                                        stop=(d == ND - 1),
                                    )
                            for ch in range(4):
                                nc.vector.tensor_scalar_add(
                                    dstT[:, ft, ch * 512 : (ch + 1) * 512],
                                    pss[ch][:],
                                    b_sb[:, ft : ft + 1],
                                )

                    proj_qk(xk_sb, wk_sb, bk_sb, kT, 1)
                    proj_qk(xq_sb, wq_sb, bq_sb, qT, 0)

                def make_vproj(pool):
                    # v projection kt-chains: row-major [keys, 256 feats]
                    def vproj(kt):
                        ops = pool.tile(
                            [128, 512], F32, tag="acc512", name=f"vp_{kt}"
                        )
                        for d in range(ND):
                            nc.tensor.matmul(
                                ops[:, 0:FPC],
                                xv_sb[:, d, kt * 128 : (kt + 1) * 128],
                                wv_sb[:, d, :],
                                start=(d == 0),
                                stop=(d == ND - 1),
                            )
                        nc.vector.tensor_add(
                            vsb[:, kt, :], ops[:, 0:FPC], bvb_sb[:]
                        )
                    return vproj

                kphase = os.environ.get("KPHASE", "full")
                if kphase == "A":
                    with (
                        tc.tile_pool(name="dbg", bufs=2) as dbg,
                        tc.tile_pool(name="psdbg", bufs=2, space="PSUM") as psd,
                    ):
                        vp = make_vproj(psd)
                        for kt in range(NM):
                            vp(kt)
                        for ft in range(NFT):
                            st = dbg.tile([128, L], F16, tag="st")
                            nc.vector.tensor_copy(st[:], qT[:, ft, :])
                            nc.sync.dma_start(
                                out_d[ft * 128 : (ft + 1) * 128, :], st[:]
                            )
                            st2 = dbg.tile([128, L], F16, tag="st")
                            nc.vector.tensor_copy(st2[:], kT[:, ft, :])
                            nc.sync.dma_start(
                                out_d[(2 + ft) * 128 : (3 + ft) * 128, :], st2[:]
                            )
                            st3 = dbg.tile([128, L], F16, tag="st")
                            nc.vector.tensor_copy(
                                st3[:], vsb[:, ft * 8 : ft * 8 + 8, :]
                            )
                            nc.sync.dma_start(
                                out_d[(4 + ft) * 128 : (5 + ft) * 128, :], st3[:]
                            )
                if kphase in ("AB", "full"):
                    phase_b(
                        nc, tc, qT, kT, vsb, ctxT, wo_sb, ones_sb, sel33,
                        d33p, out_d, kphase, make_vproj,
                    )
    nc.compile()
    return nc


def phase_b(
    nc, tc, qT, kT, vsb, ctxT, wo_sb, ones_sb, sel33, d33p, out_d, kphase,
    make_vproj,
):
    with (
        tc.tile_pool(name="probs", bufs=8) as pb,
        tc.tile_pool(name="prsum", bufs=4) as pbs,
        tc.tile_pool(name="smalls", bufs=3) as sm,
        tc.tile_pool(name="psS", bufs=2, space="PSUM") as psS,
        tc.tile_pool(name="psC", bufs=2, space="PSUM") as psC,
        tc.tile_pool(name="psD", bufs=1, space="PSUM") as psD,
        tc.tile_pool(name="psX", bufs=1, space="PSUM") as psX,
    ):
        carried = []  # deferred output-projection work items (closures)
        vproj = make_vproj(psX)

        def emit_outproj_lc(lc):
            # out^T chunk [128, 512] per ft8; contract ctxT over NFT
            for ft8 in range(D // 128):
                def work(lc=lc, ft8=ft8, pool=psX):
                    tag = "acc512" if pool is psX else (
                        "den" if pool is psD else "ctx"
                    )
                    ops = pool.tile(
                        [128, 512], F32, tag=tag, name=f"op_{lc}_{ft8}",
                    )
                    for d2 in range(NFT):
                        nc.tensor.matmul(
                            ops[:],
                            wo_sb[:, d2, ft8 * 128 : (ft8 + 1) * 128],
                            ctxT[:, d2, lc * 512 : (lc + 1) * 512],
                            start=(d2 == 0),
                            stop=(d2 == NFT - 1),
                        )
                    st = sm.tile(
                        [128, 512], F16, tag="ost", bufs=4,
                        name=f"st_{lc}_{ft8}",
                    )
                    nc.vector.tensor_copy(st[:], ops[:])
                    eng = nc.gpsimd if ft8 % 2 == 0 else nc.sync
                    eng.dma_start(
                        out_d[
                            ft8 * 128 : (ft8 + 1) * 128,
                            lc * 512 : (lc + 1) * 512,
                        ],
                        st[:],
                    )
                carried.append(work)

        for qh in range(2):
            for hp in range(2):
                den = psD.tile([128, 512], F32, tag="den", name=f"den_{qh}_{hp}")
                g0 = hp * 2  # head index base within this core's 4 heads
                for h in range(2):
                    ctx = psC.tile(
                        [128, 512], F32, tag="ctx", name=f"ctx_{qh}_{hp}_{h}"
                    )
                    q0 = qh * 1024 + h * 512
                    dp = 64 * h  # den partition base for this half
                    probs = {}
                    prsums = {}

                    def attn_consume(m, ctx=ctx, dp=dp, g0=g0, probs=probs):
                        # ctx chains: col-tiled pair, 2 heads in one bank
                        pr = probs[m]
                        for hi in range(2):
                            nc.tensor.matmul(
                                ctx[hi * 64 : (hi + 1) * 64, :],
                                vsb[:, m, (g0 + hi) * 64 : (g0 + hi + 1) * 64],
                                pr[:, hi * 512 : (hi + 1) * 512],
                                start=(m == 0),
                                stop=(m == NM - 1),
                            )
                        if not DEN_PAIR:
                            # den chains: M=1 at col positions dp, dp+32
                            for hi in range(2):
                                dr = dp + hi * 32
                                nc.tensor.matmul(
                                    den[dr : dr + 1, :],
                                    ones_sb[:],
                                    pr[:, hi * 512 : (hi + 1) * 512],
                                    start=(m == 0),
                                    stop=(m == NM - 1),
                                    tile_position=(0, dr),
                                )

                    def den_pair(p, dp=dp, prsums=prsums):
                        # denominator chains over pair-sums: M=1 cols dp, dp+32
                        ps = prsums.pop(p)
                        for hi in range(2):
                            dr = dp + hi * 32
                            nc.tensor.matmul(
                                den[dr : dr + 1, :],
                                ones_sb[:],
                                ps[:, hi * 512 : (hi + 1) * 512],
                                start=(p == 0),
                                stop=(p == NM // 2 - 1),
                                tile_position=(0, dr),
                            )

                    for m in range(NM):
                        sc = psS.tile(
                            [128, 1024], F32, tag="sc",
                            name=f"sc_{qh}_{hp}_{h}_{m}",
                        )
                        # scores: both heads, row-tiled pair (bp0 / bp64)
                        for hi in range(2):
                            po = hi * 64
                            nc.tensor.matmul(
                                sc[:, hi * 512 : (hi + 1) * 512],
                                kT[po : po + 64, hp, m * 128 : (m + 1) * 128],
                                qT[po : po + 64, hp, q0 : q0 + 512],
                                start=True,
                                stop=True,
                            )
                        # exp -> probs bf16 (both heads in one tile)
                        pr = pb.tile(
                            [128, 1024], CDT, tag="pr",
                            name=f"pr_{qh}_{hp}_{h}_{m}",
                        )
                        if _dve_tile(m):
                            nc.vector.tensor_scalar(
                                pr[:].bitcast(I16), sc[:], A16, B16, MULT, ADD
                            )
                        else:
                            nc.scalar.activation(pr[:], sc[:], EXP, scale=SCALE)
                        probs[m] = pr
                        # pair-sum probs on the idle GpSimd engine so the den
                        # matmul stream halves (den rides the pair sums)
                        if DEN_PAIR and m % 2 == 1:
                            pst = pbs.tile(
                                [128, 1024], CDT, tag="prs",
                                name=f"prs_{qh}_{hp}_{h}_{m}",
                            )
                            nc.gpsimd.tensor_add(
                                pst[:], probs[m - 1][:], probs[m][:]
                            )
                            prsums[m // 2] = pst
                        # software pipeline: consume probs LAG slots back so
                        # the PE never waits on exp (sc tiles are freed by exp,
                        # not by the chains, so this costs no PSUM)
                        if m >= LAG:
                            attn_consume(m - LAG)
                            probs.pop(m - LAG)
                        if DEN_PAIR and m >= 4 and m % 2 == 0:
                            den_pair(m // 2 - 2)
                        # carried work: v-projection (first group) feeds
                        # chains just in time (chains(m) run at slot m+LAG);
                        # outproj drains at odd m
                        if qh == 0 and hp == 0 and h == 0 and m >= 2:
                            vproj(m - 2)
                        elif POPS and carried and (m % 2) == 1:
                            carried.pop(0)()
                    if qh == 0 and hp == 0 and h == 0:
                        vproj(NM - 2)
                        vproj(NM - 1)
                    for j in range(NM - LAG, NM):
                        attn_consume(j)
                        probs.pop(j)
                    if DEN_PAIR:
                        den_pair(NM // 2 - 2)
                        den_pair(NM // 2 - 1)
                    # ---- normalize this half ----
                    # d33p rows 1-31 are memset to 1.0; only rows 0/32 carry
                    # dens, so recip never makes Inf/NaN that the selector
                    # matmul would 0*Inf-poison
                    nc.vector.tensor_copy(d33p[0:1, :], den[dp : dp + 1, :])
                    nc.vector.tensor_copy(
                        d33p[32:33, :], den[dp + 32 : dp + 33, :]
                    )
                    r33 = sm.tile(
                        [33, 512], F32, tag="r33", name=f"r33_{qh}_{hp}_{h}"
                    )
                    nc.vector.reciprocal_approx_fast(r33[:], d33p[:])
                    rb = psX.tile(
                        [128, 512], F32, tag="acc512", name=f"rb_{qh}_{hp}_{h}"
                    )
                    nc.tensor.matmul(rb[:], sel33[:], r33[:], start=True, stop=True)
                    rb_sb = sm.tile(
                        [128, 512], F32, tag="rbsb", name=f"rbsb_{qh}_{hp}_{h}"
                    )
                    nc.scalar.copy(rb_sb[:], rb[:])
                    nc.vector.tensor_mul(
                        ctxT[:, hp, q0 : q0 + 512], ctx[:], rb_sb[:]
                    )
                    # outproj chunk lc=2qh+h is complete once both hp groups
                    # normalized this half's columns
                    if hp == 1 and kphase != "AB":
                        emit_outproj_lc(qh * 2 + h)
        if kphase == "AB":
            for ft in range(NFT):
                st = sm.tile([128, L], F16, tag="ostage", bufs=1, name=f"dbg_{ft}")
                nc.vector.tensor_copy(st[:], ctxT[:, ft, :])
                nc.sync.dma_start(out_d[ft * 128 : (ft + 1) * 128, :], st[:])
            return
        # tail: remaining deferred outproj work, rotate over 4 free banks
        pools = [psX, psD, psC, psC]
        for i, work in enumerate(carried):
            work.__defaults__ = (
                work.__defaults__[0],
                work.__defaults__[1],
                pools[i % 4],
            )
            work()
        carried.clear()


def make_in_maps(Q, K, V, Wq, bq, Wk, bk, Wv, bv, Wo, bo):
    Q = np.asarray(Q, np.float32)
    K = np.asarray(K, np.float32)
    V = np.asarray(V, np.float32)
    xqT = [np.ascontiguousarray(Q[b].T).astype(NP_CDT) for b in range(B)]
    xkT = [np.ascontiguousarray(K[b].T).astype(NP_CDT) for b in range(B)]
    xvT = [np.ascontiguousarray(V[b].T).astype(NP_CDT) for b in range(B)]
    in_maps = []
    for c in range(NCORES):
        b, g = divmod(c, HPC)
        fs = slice(g * FPC, (g + 1) * FPC)
        wqT = np.ascontiguousarray(np.asarray(Wq, np.float32)[fs, :].T).astype(NP_CDT)
        wkT = np.ascontiguousarray(np.asarray(Wk, np.float32)[fs, :].T).astype(NP_CDT)
        wvT = np.ascontiguousarray(np.asarray(Wv, np.float32)[fs, :].T).astype(NP_CDT)
        woT = np.ascontiguousarray(np.asarray(Wo, np.float32)[:, fs].T).astype(NP_CDT)
        bq2 = np.ascontiguousarray(
            np.asarray(bq, np.float32)[fs].reshape(NFT, 128).T
        )
        bk2 = np.ascontiguousarray(
            np.asarray(bk, np.float32)[fs].reshape(NFT, 128).T
        )
        bv_blk = np.asarray(bv, np.float32)[fs]
        in_maps.append(
            {
                "xqT": xqT[b],
                "xkT": xkT[b],
                "xvT": xvT[b],
                "wqT": wqT,
                "wkT": wkT,
                "wvT": wvT,
                "woT": woT,
                "bq2": bq2,
                "bk2": bk2,
                "bvb": np.broadcast_to(bv_blk, (128, FPC)).copy(),
            }
        )
    return in_maps


def assemble(results, bo):
    out = np.zeros((B, L, D), np.float32)
    for c in range(NCORES):
        b = c // HPC
        out[b] += results[c][OUT_NAME].T.astype(np.float32)
    out += np.asarray(bo, np.float32)[None, None, :]
    return out


def kernel(Q, K, V, Wq, bq, Wk, bk, Wv, bv, Wo, bo):
    if "nc" not in _CACHE:
        _CACHE["nc"] = build_nc()
    nc = _CACHE["nc"]
    in_maps = make_in_maps(Q, K, V, Wq, bq, Wk, bk, Wv, bv, Wo, bo)
    res = run_bass_kernel_spmd(nc, in_maps, core_ids=list(range(NCORES)))
    return assemble(res.results, bo)
